# revision 4
# baseline (speedup 1.0000x reference)
"""Trainium2 Bass kernel: sparse (masked) attention with L2 row-normalization.

Per batch b (reference semantics, fp32):
    q = x @ Wq.T ; k = x @ Wk.T ; v = x @ Wv.T          # x: [N, D]
    rel[n, m] = (q[n] . k[m]) * adjacency[m, n]          # multiplicative mask
    out[n]    = sum_m rel[n, m] / ||rel[n, :]||_2 * v[m]

Sharding: data-parallel over batch B=8 -> one batch per NeuronCore, no
collectives. adjacency/weights replicated.

Per-core layout strategy (all matmul operands bf16, fp32 PSUM accumulate):
  - host supplies x^T [D, N] so every contraction dim is a partition dim
  - scores are computed transposed: S^T[m, n] = sum_e kT[e,m] qT[e,n], so
    the mask is adjacency in its NATIVE layout and the AV matmul needs no
    transposes (lhsT = S^T tile, rhs = v tile)
  - row sum-of-squares (a partition-dim reduction) via ones-vector matmuls
    accumulated in PSUM across m-tiles
  - 1/||row|| applied as a per-partition scalar on the AV output tiles
"""

from contextlib import ExitStack

import numpy as np
import ml_dtypes

B, N, D = 8, 2048, 512
P = 128  # SBUF partitions
CHUNK = 512  # fp32 free-dim elems per PSUM bank

_cached = {}


def _build(n=N, d=D):
    import concourse.bacc as bacc
    import concourse.mybir as mybir
    import concourse.tile as tile

    f32 = mybir.dt.float32
    bf16 = mybir.dt.bfloat16

    nt = n // P  # key/query 128-tiles
    dt = d // P  # feature 128-tiles
    ch = min(CHUNK, n)  # free-dim chunk size
    nch = n // ch  # chunks over n
    tpc = ch // P  # 128-tiles per chunk

    nc = bacc.Bacc("TRN2", target_bir_lowering=False, debug=False, num_devices=B)

    xT_h = nc.dram_tensor("xT", [d, n], bf16, kind="ExternalInput")
    adj_h = nc.dram_tensor("adj", [n, n], bf16, kind="ExternalInput")
    wqT_h = nc.dram_tensor("wqT", [d, d], bf16, kind="ExternalInput")
    wkT_h = nc.dram_tensor("wkT", [d, d], bf16, kind="ExternalInput")
    wvT_h = nc.dram_tensor("wvT", [d, d], bf16, kind="ExternalInput")
    out_h = nc.dram_tensor("out", [n, d], f32, kind="ExternalOutput")

    with tile.TileContext(nc) as tc, ExitStack() as ctx:
        sb = ctx.enter_context(tc.tile_pool(name="sb", bufs=1))
        adj_pool = ctx.enter_context(tc.tile_pool(name="adjp", bufs=3))
        sq_pool = ctx.enter_context(tc.tile_pool(name="sqp", bufs=3))
        outp = ctx.enter_context(tc.tile_pool(name="outp", bufs=2))
        psum = ctx.enter_context(tc.tile_pool(name="psum", bufs=4, space="PSUM"))
        pnrm_pool = ctx.enter_context(tc.tile_pool(name="pnrm", bufs=1, space="PSUM"))

        # ---- weight + xT loads -------------------------------------------
        w_sb = {}
        for wname, wh in (("wq", wqT_h), ("wk", wkT_h), ("wv", wvT_h)):
            for e in range(dt):
                t = sb.tile([P, d], bf16, name=f"{wname}{e}", tag=f"{wname}{e}")
                nc.sync.dma_start(t[:], wh[e * P : (e + 1) * P, :])
                w_sb[wname, e] = t

        xT_sb = []
        for e in range(dt):
            t = sb.tile([P, n], bf16, name=f"xT{e}", tag=f"xT{e}")
            for c in range(nch):
                nc.sync.dma_start(
                    t[:, c * ch : (c + 1) * ch],
                    xT_h[e * P : (e + 1) * P, c * ch : (c + 1) * ch],
                )
            xT_sb.append(t)

        ones = sb.tile([P, 1], bf16, name="ones", tag="ones")
        nc.vector.memset(ones[:], 1.0)

        # ---- projections --------------------------------------------------
        # qT/kT [e, n] (partition = feature), accumulated over input-dim tiles
        qT_sb, kT_sb = [], []
        for pname, store in (("wq", qT_sb), ("wk", kT_sb)):
            for e in range(dt):
                t = sb.tile([P, n], bf16, name=f"{pname}T{e}", tag=f"{pname}T{e}")
                store.append(t)
                for c in range(nch):
                    pt = psum.tile([P, ch], f32, name="mm", tag="mm")
                    for dd in range(dt):
                        nc.tensor.matmul(
                            pt[:],
                            w_sb[pname, dd][:, e * P : (e + 1) * P],
                            xT_sb[dd][:, c * ch : (c + 1) * ch],
                            start=(dd == 0),
                            stop=(dd == dt - 1),
                        )
                    nc.vector.tensor_copy(t[:, c * ch : (c + 1) * ch], pt[:])

        # v [m, d] (partition = key index)
        v_sb = []
        for m in range(nt):
            t = sb.tile([P, d], bf16, name=f"v{m}", tag=f"v{m}")
            v_sb.append(t)
            pt = psum.tile([P, d], f32, name="mm", tag="mm")
            for e in range(dt):
                nc.tensor.matmul(
                    pt[:],
                    xT_sb[e][:, m * P : (m + 1) * P],
                    w_sb["wv", e][:],
                    start=(e == 0),
                    stop=(e == dt - 1),
                )
            nc.vector.tensor_copy(t[:], pt[:])

        # ---- scores + mask + sum-of-squares -------------------------------
        st_sb = [sb.tile([P, n], bf16, name=f"st{m}", tag=f"st{m}") for m in range(nt)]
        pnrm = [pnrm_pool.tile([1, ch], f32, name=f"pnrm{c}", tag=f"pnrm{c}") for c in range(nch)]

        for m in range(nt):
            adj_t = adj_pool.tile([P, n], bf16, name="adj_t", tag="adj_t")
            nc.sync.dma_start(adj_t[:], adj_h[m * P : (m + 1) * P, :])
            for c in range(nch):
                ps = psum.tile([P, ch], f32, name="mm", tag="mm")
                for e in range(dt):
                    nc.tensor.matmul(
                        ps[:],
                        kT_sb[e][:, m * P : (m + 1) * P],
                        qT_sb[e][:, c * ch : (c + 1) * ch],
                        start=(e == 0),
                        stop=(e == dt - 1),
                    )
                stm = st_sb[m][:, c * ch : (c + 1) * ch]
                nc.vector.tensor_mul(stm, ps[:], adj_t[:, c * ch : (c + 1) * ch])
                sq = sq_pool.tile([P, ch], bf16, name="sq", tag="sq")
                nc.scalar.square(sq[:], stm)
                nc.tensor.matmul(
                    pnrm[c][:],
                    ones[:],
                    sq[:],
                    start=(m == 0),
                    stop=(m == nt - 1),
                )

        # ---- 1/||row||: sqrt (ACT) -> reciprocal (DVE) -> [P, nt] layout --
        rcp_t = sb.tile([P, nt], f32, name="rcp_t", tag="rcp_t")
        for c in range(nch):
            nrm_row = sq_pool.tile([1, ch], f32, name="nrm_row", tag="nrm_row")
            nc.scalar.sqrt(nrm_row[:], pnrm[c][:])
            rcp_row = sq_pool.tile([1, ch], f32, name="rcp_row", tag="rcp_row")
            nc.vector.reciprocal(rcp_row[:], nrm_row[:])
            # scatter [1, ch] -> [P, tpc] so scale is a per-partition scalar
            for tt in range(tpc):
                nc.sync.dma_start(
                    rcp_t[:, c * tpc + tt : c * tpc + tt + 1],
                    rcp_row[:, tt * P : (tt + 1) * P],
                )

        # ---- AV + normalization scale ------------------------------------
        for t in range(nt):
            pav = psum.tile([P, d], f32, name="mm", tag="mm")
            for m in range(nt):
                nc.tensor.matmul(
                    pav[:],
                    st_sb[m][:, t * P : (t + 1) * P],
                    v_sb[m][:],
                    start=(m == 0),
                    stop=(m == nt - 1),
                )
            ot = outp.tile([P, d], f32, name="ot", tag="ot")
            nc.vector.tensor_scalar_mul(ot[:], pav[:], rcp_t[:, t : t + 1])
            nc.sync.dma_start(out_h[t * P : (t + 1) * P, :], ot[:])

    nc.compile()
    return nc


def _prep_in_maps(inputs):
    bf = ml_dtypes.bfloat16
    x = np.asarray(inputs["neuron_states"])
    adj = np.ascontiguousarray(np.asarray(inputs["adjacency"]).astype(bf))
    wqT = np.ascontiguousarray(np.asarray(inputs["Wq"]).T.astype(bf))
    wkT = np.ascontiguousarray(np.asarray(inputs["Wk"]).T.astype(bf))
    wvT = np.ascontiguousarray(np.asarray(inputs["Wv"]).T.astype(bf))
    in_maps = []
    for b in range(x.shape[0]):
        xT = np.ascontiguousarray(x[b].T.astype(bf))
        in_maps.append({"xT": xT, "adj": adj, "wqT": wqT, "wkT": wkT, "wvT": wvT})
    return in_maps


def _run(inputs, trace=False, **kw):
    from concourse.bass_utils import run_bass_kernel_spmd

    if "nc" not in _cached:
        _cached["nc"] = _build()
    in_maps = _prep_in_maps(inputs)
    res = run_bass_kernel_spmd(
        _cached["nc"], in_maps, core_ids=list(range(len(in_maps))), trace=trace, **kw
    )
    out = np.stack([r["out"] for r in res.results], axis=0)
    return out, res


def kernel(**inputs):
    return _run(inputs)[0]


# revision 6
# speedup vs baseline: 1.0337x; 1.0337x over previous
"""Trainium2 Bass kernel: sparse (masked) attention with L2 row-normalization.

Per batch b (reference semantics, fp32):
    q = x @ Wq.T ; k = x @ Wk.T ; v = x @ Wv.T          # x: [N, D]
    rel[n, m] = (q[n] . k[m]) * adjacency[m, n]          # multiplicative mask
    out[n]    = sum_m rel[n, m] / ||rel[n, :]||_2 * v[m]

Sharding: data-parallel over batch B=8 -> one batch per NeuronCore, no
collectives. adjacency/weights replicated.

Per-core layout strategy (all matmul operands bf16, fp32 PSUM accumulate):
  - host supplies x^T [D, N] so every contraction dim is a partition dim
  - scores are computed transposed: S^T[m, n] = sum_e kT[e,m] qT[e,n], so
    the mask is adjacency in its NATIVE layout and the AV matmul needs no
    transposes (lhsT = S^T tile, rhs = v tile)
  - row sum-of-squares (a partition-dim reduction) via ones-vector matmuls
    accumulated in PSUM across m-tiles; emission delayed so the
    mask(DVE) -> square(ACT) -> matmul(PE) chain never stalls the PE
  - 1/||row|| applied as a per-partition scalar on the AV output tiles
"""

from contextlib import ExitStack

import numpy as np
import ml_dtypes

B, N, D = 8, 2048, 512
P = 128  # SBUF partitions
CHUNK = 512  # fp32 free-dim elems per PSUM bank

_cached = {}


def _build(n=N, d=D):
    import concourse.bacc as bacc
    import concourse.mybir as mybir
    import concourse.tile as tile

    f32 = mybir.dt.float32
    bf16 = mybir.dt.bfloat16

    nt = n // P  # key/query 128-tiles
    dt = d // P  # feature 128-tiles
    ch = min(CHUNK, n)  # free-dim chunk size
    nch = n // ch  # chunks over n
    tpc = ch // P  # 128-tiles per chunk

    nc = bacc.Bacc("TRN2", target_bir_lowering=False, debug=False, num_devices=B)

    xT_h = nc.dram_tensor("xT", [d, n], bf16, kind="ExternalInput")
    adj_h = nc.dram_tensor("adj", [n, n], bf16, kind="ExternalInput")
    wqT_h = nc.dram_tensor("wqT", [d, d], bf16, kind="ExternalInput")
    wkT_h = nc.dram_tensor("wkT", [d, d], bf16, kind="ExternalInput")
    wvT_h = nc.dram_tensor("wvT", [d, d], bf16, kind="ExternalInput")
    out_h = nc.dram_tensor("out", [n, d], f32, kind="ExternalOutput")

    with tile.TileContext(nc) as tc, ExitStack() as ctx:
        sb = ctx.enter_context(tc.tile_pool(name="sb", bufs=1))
        adj_pool = ctx.enter_context(tc.tile_pool(name="adjp", bufs=3))
        sq_pool = ctx.enter_context(tc.tile_pool(name="sqp", bufs=4))
        outp = ctx.enter_context(tc.tile_pool(name="outp", bufs=2))
        psum = ctx.enter_context(tc.tile_pool(name="psum", bufs=4, space="PSUM"))
        pnrm_pool = ctx.enter_context(tc.tile_pool(name="pnrm", bufs=1, space="PSUM"))

        # ---- input loads: one batched DMA per weight, per-chunk for xT ----
        # order: wq -> xT -> wk -> wv so the first projection matmul can
        # start as early as possible
        wq_sb = sb.tile([P, dt, d], bf16, name="wq_sb", tag="wq_sb")
        nc.sync.dma_start(wq_sb[:], wqT_h.rearrange("(t p) e -> p t e", p=P))

        xT_sb = sb.tile([P, dt, n], bf16, name="xT_sb", tag="xT_sb")
        xT_r = xT_h.rearrange("(t p) n -> p t n", p=P)
        nc.sync.dma_start(xT_sb[:, :, 0:ch], xT_r[:, :, 0:ch])
        for c in range(1, nch):
            nc.scalar.dma_start(
                xT_sb[:, :, c * ch : (c + 1) * ch], xT_r[:, :, c * ch : (c + 1) * ch]
            )

        wk_sb = sb.tile([P, dt, d], bf16, name="wk_sb", tag="wk_sb")
        nc.scalar.dma_start(wk_sb[:], wkT_h.rearrange("(t p) e -> p t e", p=P))
        wv_sb = sb.tile([P, dt, d], bf16, name="wv_sb", tag="wv_sb")
        nc.scalar.dma_start(wv_sb[:], wvT_h.rearrange("(t p) e -> p t e", p=P))
        w_sb = {"wq": wq_sb, "wk": wk_sb, "wv": wv_sb}

        ones = sb.tile([P, 1], bf16, name="ones", tag="ones")
        nc.vector.memset(ones[:], 1.0)

        # ---- projections --------------------------------------------------
        # qT/kT [e, n] (partition = feature), accumulated over input-dim tiles
        qT_sb, kT_sb = [], []
        for pname, store in (("wq", qT_sb), ("wk", kT_sb)):
            for e in range(dt):
                t = sb.tile([P, n], bf16, name=f"{pname}T{e}", tag=f"{pname}T{e}")
                store.append(t)
                for c in range(nch):
                    pt = psum.tile([P, ch], f32, name="mm", tag="mm")
                    for dd in range(dt):
                        nc.tensor.matmul(
                            pt[:],
                            w_sb[pname][:, dd, e * P : (e + 1) * P],
                            xT_sb[:, dd, c * ch : (c + 1) * ch],
                            start=(dd == 0),
                            stop=(dd == dt - 1),
                        )
                    nc.vector.tensor_copy(t[:, c * ch : (c + 1) * ch], pt[:])

        # v [m, d] (partition = key index)
        v_sb = []
        for m in range(nt):
            t = sb.tile([P, d], bf16, name=f"v{m}", tag=f"v{m}")
            v_sb.append(t)
            pt = psum.tile([P, d], f32, name="mm", tag="mm")
            for e in range(dt):
                nc.tensor.matmul(
                    pt[:],
                    xT_sb[:, e, m * P : (m + 1) * P],
                    wv_sb[:, e, :],
                    start=(e == 0),
                    stop=(e == dt - 1),
                )
            nc.vector.tensor_copy(t[:], pt[:])

        # ---- scores + mask + sum-of-squares -------------------------------
        st_sb = [sb.tile([P, n], bf16, name=f"st{m}", tag=f"st{m}") for m in range(nt)]
        pnrm = [
            pnrm_pool.tile([1, ch], f32, name=f"pnrm{c}", tag=f"pnrm{c}")
            for c in range(nch)
        ]

        # norm matmuls are emitted DELAY (m,c)-units after their square so the
        # PE never waits on the DVE/ACT chain
        pending = []
        nrm_count = [0] * nch

        def emit_norm(limit):
            while len(pending) > limit:
                c, sq_t = pending.pop(0)
                nc.tensor.matmul(
                    pnrm[c][:],
                    ones[:],
                    sq_t[:],
                    start=(nrm_count[c] == 0),
                    stop=(nrm_count[c] == nt - 1),
                )
                nrm_count[c] += 1

        for m in range(nt):
            adj_t = adj_pool.tile([P, n], bf16, name="adj_t", tag="adj_t")
            nc.gpsimd.dma_start(adj_t[:], adj_h[m * P : (m + 1) * P, :])
            for c in range(nch):
                ps = psum.tile([P, ch], f32, name="mm", tag="mm")
                for e in range(dt):
                    nc.tensor.matmul(
                        ps[:],
                        kT_sb[e][:, m * P : (m + 1) * P],
                        qT_sb[e][:, c * ch : (c + 1) * ch],
                        start=(e == 0),
                        stop=(e == dt - 1),
                    )
                stm = st_sb[m][:, c * ch : (c + 1) * ch]
                nc.vector.tensor_mul(stm, ps[:], adj_t[:, c * ch : (c + 1) * ch])
                sq = sq_pool.tile([P, ch], bf16, name="sq", tag="sq")
                nc.scalar.square(sq[:], stm)
                pending.append((c, sq))
                emit_norm(2)
        emit_norm(0)

        # ---- 1/||row||: sqrt rows (ACT) -> scatter -> one [P, nt] recip ---
        nrm_t = sb.tile([P, nt], f32, name="nrm_t", tag="nrm_t")
        for c in range(nch):
            nrm_row = sq_pool.tile([1, ch], f32, name="nrm_row", tag="nrm_row")
            nc.scalar.sqrt(nrm_row[:], pnrm[c][:])
            # scatter [1, ch] -> [P, tpc] so scale is a per-partition scalar
            for tt in range(tpc):
                nc.gpsimd.dma_start(
                    nrm_t[:, c * tpc + tt : c * tpc + tt + 1],
                    nrm_row[:, tt * P : (tt + 1) * P],
                )
        rcp_t = sb.tile([P, nt], f32, name="rcp_t", tag="rcp_t")
        nc.vector.reciprocal(rcp_t[:], nrm_t[:])

        # ---- AV + normalization scale ------------------------------------
        for t in range(nt):
            pav = psum.tile([P, d], f32, name="mm", tag="mm")
            for m in range(nt):
                nc.tensor.matmul(
                    pav[:],
                    st_sb[m][:, t * P : (t + 1) * P],
                    v_sb[m][:],
                    start=(m == 0),
                    stop=(m == nt - 1),
                )
            ot = outp.tile([P, d], f32, name="ot", tag="ot")
            nc.vector.tensor_scalar_mul(ot[:], pav[:], rcp_t[:, t : t + 1])
            nc.scalar.dma_start(out_h[t * P : (t + 1) * P, :], ot[:])

    nc.compile()
    return nc


def _prep_in_maps(inputs):
    bf = ml_dtypes.bfloat16
    x = np.asarray(inputs["neuron_states"])
    adj = np.ascontiguousarray(np.asarray(inputs["adjacency"]).astype(bf))
    wqT = np.ascontiguousarray(np.asarray(inputs["Wq"]).T.astype(bf))
    wkT = np.ascontiguousarray(np.asarray(inputs["Wk"]).T.astype(bf))
    wvT = np.ascontiguousarray(np.asarray(inputs["Wv"]).T.astype(bf))
    in_maps = []
    for b in range(x.shape[0]):
        xT = np.ascontiguousarray(x[b].T.astype(bf))
        in_maps.append({"xT": xT, "adj": adj, "wqT": wqT, "wkT": wkT, "wvT": wvT})
    return in_maps


def _run(inputs, trace=False, **kw):
    from concourse.bass_utils import run_bass_kernel_spmd

    if "nc" not in _cached:
        _cached["nc"] = _build()
    in_maps = _prep_in_maps(inputs)
    res = run_bass_kernel_spmd(
        _cached["nc"], in_maps, core_ids=list(range(len(in_maps))), trace=trace, **kw
    )
    out = np.stack([r["out"] for r in res.results], axis=0)
    return out, res


def kernel(**inputs):
    return _run(inputs)[0]


# revision 7
# speedup vs baseline: 1.0952x; 1.0595x over previous
"""Trainium2 Bass kernel: sparse (masked) attention with L2 row-normalization.

Per batch b (reference semantics, fp32):
    q = x @ Wq.T ; k = x @ Wk.T ; v = x @ Wv.T          # x: [N, D]
    rel[n, m] = (q[n] . k[m]) * adjacency[m, n]          # multiplicative mask
    out[n]    = sum_m rel[n, m] / ||rel[n, :]||_2 * v[m]

Sharding: data-parallel over batch B=8 -> one batch per NeuronCore, no
collectives. adjacency/weights replicated.

Per-core layout strategy (all matmul operands bf16, fp32 PSUM accumulate):
  - host prepacks x^T and the (transposed) weights into partition-major,
    fully contiguous layouts so input DMAs run at full HBM bandwidth
  - scores are computed transposed: S^T[m, n] = sum_e kT[e,m] qT[e,n], so
    the mask is adjacency in its NATIVE layout and the AV matmul needs no
    transposes (lhsT = S^T tile, rhs = v tile)
  - row sum-of-squares (a partition-dim reduction) via ones-vector matmuls
    accumulated in PSUM across m-tiles; the 4 chunk-norm matmuls of an
    m-tile are emitted as one batch, one m-tile late, so the single
    ones-LDWEIGHTS and the DVE/ACT chain never stall the PE pipeline
  - 1/||row|| applied as a per-partition scalar on the AV output tiles
"""

from contextlib import ExitStack

import numpy as np
import ml_dtypes

B, N, D = 8, 2048, 512
P = 128  # SBUF partitions
CHUNK = 512  # fp32 free-dim elems per PSUM bank

_cached = {}


def _build(n=N, d=D):
    import concourse.bacc as bacc
    import concourse.mybir as mybir
    import concourse.tile as tile

    f32 = mybir.dt.float32
    bf16 = mybir.dt.bfloat16

    nt = n // P  # key/query 128-tiles
    dt = d // P  # feature 128-tiles
    ch = min(CHUNK, n)  # free-dim chunk size
    nch = n // ch  # chunks over n
    tpc = ch // P  # 128-tiles per chunk

    nc = bacc.Bacc("TRN2", target_bir_lowering=False, debug=False, num_devices=B)

    # host-prepacked: xTp[p, c, t, j] = x.T[t*P+p, c*ch+j]
    xT_h = nc.dram_tensor("xTp", [P, nch, dt, ch], bf16, kind="ExternalInput")
    # host-prepacked: w3[p, i, t, e] = W_i.T[t*P+p, e], i in (q, k, v)
    w3_h = nc.dram_tensor("w3", [P, 3, dt, d], bf16, kind="ExternalInput")
    adj_h = nc.dram_tensor("adj", [n, n], bf16, kind="ExternalInput")
    out_h = nc.dram_tensor("out", [n, d], f32, kind="ExternalOutput")

    with tile.TileContext(nc) as tc, ExitStack() as ctx:
        sb = ctx.enter_context(tc.tile_pool(name="sb", bufs=1))
        adj_pool = ctx.enter_context(tc.tile_pool(name="adjp", bufs=3))
        sq_pool = ctx.enter_context(tc.tile_pool(name="sqp", bufs=8))
        outp = ctx.enter_context(tc.tile_pool(name="outp", bufs=2))
        psum = ctx.enter_context(tc.tile_pool(name="psum", bufs=4, space="PSUM"))
        pnrm_pool = ctx.enter_context(tc.tile_pool(name="pnrm", bufs=1, space="PSUM"))

        # ---- input loads (all fully contiguous per partition) -------------
        # order: wq -> xT chunk 0 -> rest, so the first matmul starts early
        wq_sb = sb.tile([P, dt, d], bf16, name="wq_sb", tag="wq_sb")
        nc.sync.dma_start(wq_sb[:], w3_h[:, 0])

        xT_sb = sb.tile([P, nch, dt, ch], bf16, name="xT_sb", tag="xT_sb")
        nc.sync.dma_start(xT_sb[:, 0], xT_h[:, 0])
        for c in range(1, nch):
            nc.scalar.dma_start(xT_sb[:, c], xT_h[:, c])

        wk_sb = sb.tile([P, dt, d], bf16, name="wk_sb", tag="wk_sb")
        nc.scalar.dma_start(wk_sb[:], w3_h[:, 1])
        wv_sb = sb.tile([P, dt, d], bf16, name="wv_sb", tag="wv_sb")
        nc.scalar.dma_start(wv_sb[:], w3_h[:, 2])
        w_sb = {"wq": wq_sb, "wk": wk_sb, "wv": wv_sb}

        ones = sb.tile([P, 1], bf16, name="ones", tag="ones")
        nc.vector.memset(ones[:], 1.0)

        # ---- projections --------------------------------------------------
        # qT/kT [e, n] (partition = feature), accumulated over input-dim tiles
        qT_sb, kT_sb = [], []
        for pname, store in (("wq", qT_sb), ("wk", kT_sb)):
            for e in range(dt):
                t = sb.tile([P, n], bf16, name=f"{pname}T{e}", tag=f"{pname}T{e}")
                store.append(t)
                for c in range(nch):
                    pt = psum.tile([P, ch], f32, name="mm", tag="mm")
                    for dd in range(dt):
                        nc.tensor.matmul(
                            pt[:],
                            w_sb[pname][:, dd, e * P : (e + 1) * P],
                            xT_sb[:, c, dd, :],
                            start=(dd == 0),
                            stop=(dd == dt - 1),
                        )
                    nc.vector.tensor_copy(t[:, c * ch : (c + 1) * ch], pt[:])

        # v [m, d] (partition = key index)
        v_sb = []
        for m in range(nt):
            t = sb.tile([P, d], bf16, name=f"v{m}", tag=f"v{m}")
            v_sb.append(t)
            pt = psum.tile([P, d], f32, name="mm", tag="mm")
            for e in range(dt):
                nc.tensor.matmul(
                    pt[:],
                    xT_sb[:, m // tpc, e, (m % tpc) * P : (m % tpc + 1) * P],
                    wv_sb[:, e, :],
                    start=(e == 0),
                    stop=(e == dt - 1),
                )
            nc.vector.tensor_copy(t[:], pt[:])

        # ---- scores + mask + sum-of-squares -------------------------------
        st_sb = [sb.tile([P, n], bf16, name=f"st{m}", tag=f"st{m}") for m in range(nt)]
        pnrm = [
            pnrm_pool.tile([1, ch], f32, name=f"pnrm{c}", tag=f"pnrm{c}")
            for c in range(nch)
        ]

        # norm matmuls for m-tile m are emitted as one batch (single
        # ones-LDWEIGHTS) after m+1's scores, so the PE pipeline never breaks
        sq_tiles = {}

        def emit_norm_batch(mm_idx):
            for c in range(nch):
                nc.tensor.matmul(
                    pnrm[c][:],
                    ones[:],
                    sq_tiles.pop((mm_idx, c))[:],
                    start=(mm_idx == 0),
                    stop=(mm_idx == nt - 1),
                )

        for m in range(nt):
            adj_t = adj_pool.tile([P, n], bf16, name="adj_t", tag="adj_t")
            nc.gpsimd.dma_start(adj_t[:], adj_h[m * P : (m + 1) * P, :])
            for c in range(nch):
                ps = psum.tile([P, ch], f32, name="mm", tag="mm")
                for e in range(dt):
                    nc.tensor.matmul(
                        ps[:],
                        kT_sb[e][:, m * P : (m + 1) * P],
                        qT_sb[e][:, c * ch : (c + 1) * ch],
                        start=(e == 0),
                        stop=(e == dt - 1),
                    )
                stm = st_sb[m][:, c * ch : (c + 1) * ch]
                nc.vector.tensor_mul(stm, ps[:], adj_t[:, c * ch : (c + 1) * ch])
                sq = sq_pool.tile([P, ch], bf16, name="sq", tag="sq")
                nc.scalar.square(sq[:], stm)
                sq_tiles[m, c] = sq
            if m >= 1:
                emit_norm_batch(m - 1)
        emit_norm_batch(nt - 1)

        # ---- 1/||row||: sqrt rows (ACT) -> scatter -> one [P, nt] recip ---
        nrm_t = sb.tile([P, nt], f32, name="nrm_t", tag="nrm_t")
        for c in range(nch):
            nrm_row = sq_pool.tile([1, ch], f32, name="nrm_row", tag="nrm_row")
            nc.scalar.sqrt(nrm_row[:], pnrm[c][:])
            # scatter [1, ch] -> [P, tpc] so scale is a per-partition scalar
            for tt in range(tpc):
                nc.gpsimd.dma_start(
                    nrm_t[:, c * tpc + tt : c * tpc + tt + 1],
                    nrm_row[:, tt * P : (tt + 1) * P],
                )
        rcp_t = sb.tile([P, nt], f32, name="rcp_t", tag="rcp_t")
        nc.vector.reciprocal(rcp_t[:], nrm_t[:])

        # ---- AV + normalization scale ------------------------------------
        for t in range(nt):
            pav = psum.tile([P, d], f32, name="mm", tag="mm")
            for m in range(nt):
                nc.tensor.matmul(
                    pav[:],
                    st_sb[m][:, t * P : (t + 1) * P],
                    v_sb[m][:],
                    start=(m == 0),
                    stop=(m == nt - 1),
                )
            ot = outp.tile([P, d], f32, name="ot", tag="ot")
            nc.vector.tensor_scalar_mul(ot[:], pav[:], rcp_t[:, t : t + 1])
            nc.scalar.dma_start(out_h[t * P : (t + 1) * P, :], ot[:])

    nc.compile()
    return nc


def _prep_in_maps(inputs, n=N, d=D):
    bf = ml_dtypes.bfloat16
    nt = n // P
    dt = d // P
    ch = min(CHUNK, n)
    nch = n // ch

    x = np.asarray(inputs["neuron_states"])
    adj = np.ascontiguousarray(np.asarray(inputs["adjacency"]).astype(bf))
    w_all = np.stack(
        [np.asarray(inputs["Wq"]).T, np.asarray(inputs["Wk"]).T, np.asarray(inputs["Wv"]).T]
    ).astype(bf)
    # w3[p, i, t, e] = W_i.T[t*P+p, e]
    w3 = np.ascontiguousarray(w_all.reshape(3, dt, P, d).transpose(2, 0, 1, 3))
    in_maps = []
    for b in range(x.shape[0]):
        xT = x[b].T.astype(bf)  # [d, n]
        # xTp[p, c, t, j] = xT[t*P+p, c*ch+j]
        xTp = np.ascontiguousarray(
            xT.reshape(dt, P, nch, ch).transpose(1, 2, 0, 3)
        )
        in_maps.append({"xTp": xTp, "adj": adj, "w3": w3})
    return in_maps


def _run(inputs, trace=False, **kw):
    from concourse.bass_utils import run_bass_kernel_spmd

    if "nc" not in _cached:
        _cached["nc"] = _build()
    in_maps = _prep_in_maps(inputs)
    res = run_bass_kernel_spmd(
        _cached["nc"], in_maps, core_ids=list(range(len(in_maps))), trace=trace, **kw
    )
    out = np.stack([r["out"] for r in res.results], axis=0)
    return out, res


def kernel(**inputs):
    return _run(inputs)[0]
